# revision 40
# baseline (speedup 1.0000x reference)
"""Trainium2 Bass kernel for nn_LinearAttention (RoPE(Q) @ RoPE(Q)^T @ V).

Algebraic core: no softmax, so out = (QR@QR^T)@V == QR@(QR^T@V) with a
128x128 per-head-pair intermediate. 16 heads / 8 cores = 2 heads per
core; the two heads ride the two 64-wide lanes of the 128x128 PE array.

Layout: t = p*16 + (r*8 + c) (p = SBUF partition, r = range 0/1,
c = chunk-in-range); the host packs/unpacks with this permutation.

Schedule (from NTFF trace + probe measurements):
  - DMA: under 8-core HBM contention each HWDGE queue sustains only
    ~130-160 GB/s, so per-queue FIFO order is the priority mechanism.
    ACT ring: QQ0 (the RoPE gate, alone at the head), QQ1, V-hi.
    SP ring: tables r0, tables r1 (+identity), V-lo. All 6 issues are
    hoisted (BIR surgery) before the bass-init barrier so transfers
    stream during the tail of the fixed ~5.9us runtime preamble.
    Measured dead ends: gpsimd SWDGE V (first-byte latency + steals
    early packets from QQ0), partition-half splits (packets spray
    across all 16 SDMA engines and fight for ports), sub-[P,1024]
    pieces (1KB packets halve the queue rate).
  - RoPE runs ONLY on DVE (probe: a concurrent GpSimd tensor op slows
    DVE ~3.3x via SBUF port contention; GpSimd TT is 2.7x slower than
    DVE anyway). 6 ops per range at FD=512; r1's combines split in
    half so the S-matmuls for chunks 8-11 start one combine earlier.
  - qr rows are (h, x, kh) so each head's S2 block is contiguous: the
    head-diagonal s2d selection is 2 rectangle copies (DVE+ACT in
    parallel), not 4 strip copies.
  - Phase 2 PE order: transposes c0-7 (need only qr), S-matmuls c0-15
    (V lands in time), transposes c8-15, phase 3. Transposes use
    is_transpose (bf16 PSUM = half the evac bytes); c0-7 evac on ACT
    early, c8-15 evacs on DVE (bf16 copy is 2x faster there). qrt
    lives in 3 tiles to avoid tile-granular false deps into phase 3.
  - Phase 3: 4 independent 512-col matmuls off the single s2d weight;
    per-block casts split DVE(lo)/ACT(hi), except g3-hi on DVE so ACT
    is free to issue the final out DMA immediately. Outputs leave as
    two 1024-col (2KB-row) DMAs: blocks 0+1 on SP, 2+3 on ACT.
  - kernel tail: per-engine drains + a sem-only barrier; the final
    drain waits only on the DMA semaphore lanes.
"""

from contextlib import ExitStack

import numpy as np

import concourse.bass as bass
import concourse.mybir as mybir
import concourse.tile as tile
from concourse.bass_utils import run_bass_kernel_spmd
from concourse.vector_clock import ScopedClock

H, T, D = 16, 2048, 64
N_CORES = 8
HPC = H // N_CORES  # heads per core
P = 128
NT = T // P  # 16 t-chunks per head
HD = D // 2
F32 = mybir.dt.float32
BF16 = mybir.dt.bfloat16


def _rope_tables():
    inv_freq = 1.0 / (10000.0 ** (np.arange(0, D, 2, dtype=np.float32) / D))
    t = np.arange(T, dtype=np.float32)
    freqs = np.outer(t, inv_freq).astype(np.float32)  # [T, D/2]
    return np.cos(freqs).astype(np.float32), np.sin(freqs).astype(np.float32)


class _SlimTileContext(tile.TileContext):
    """TileContext whose kernel tail uses per-engine drains + a
    sequencer-level (sem-only) barrier instead of the full EVSEM
    butterfly."""

    def _drain_and_barrier(self, tick_clock, wait_clock):
        nc = self.nc
        drain_inst = nc.sync.drain()
        # Engines are ordered by the all_engine_barrier below (engine ops
        # complete in queue order); only ASYNC DMA transfers need the
        # final sem waits, so filter the clock to the DMA lanes.
        import re

        import bass_rust as _br

        gclk = tick_clock.global_clock
        vals = [int(x) for x in re.findall(r"\d+", repr(gclk))]
        dma_clock = _br.VectorClock()
        for idx, name in enumerate(_br.PROC_NAMES):
            if "DMA" in name:
                for _ in range(vals[idx]):
                    dma_clock.advance(idx)
        wait_clock.add_sem_waits(drain_inst.ins, ScopedClock({None: dma_clock}))
        for eng in nc.engines.values():
            if eng.engine != mybir.EngineType.SP:
                eng.drain(fusable=False)
        nc.all_engine_barrier(sem_only=True)
        popped = nc._tile_sem_poison_stack.pop()
        assert popped is self._sem_poison
        nc.clear_and_free_semaphores(list(self.sems.allocated().values()))


def _build_nc():
    nc = bass.Bass()
    # Under 8-core HBM contention each HWDGE queue sustains ~130-160
    # GB/s regardless of packet size (partition-half splits are WORSE:
    # packets spread over all 16 SDMA engines and fight for ports).
    # So: per-queue FIFO order is the priority mechanism, and the
    # critical piece (Q-r0) rides alone at the head of one queue.
    QQ0 = nc.declare_dram_parameter("QQ0", [P, 2 * 8 * HPC * HD], BF16, isOutput=False)
    QQ1 = nc.declare_dram_parameter("QQ1", [P, 2 * 8 * HPC * HD], BF16, isOutput=False)
    VV0 = nc.declare_dram_parameter("VV0", [P, 8 * HPC * D], BF16, isOutput=False)
    VV1 = nc.declare_dram_parameter("VV1", [P, 8 * HPC * D], BF16, isOutput=False)
    TB0 = nc.declare_dram_parameter("TB0", [P, 2 * 8 * HD + P], BF16, isOutput=False)
    TB1 = nc.declare_dram_parameter("TB1", [P, 2 * 8 * HD], BF16, isOutput=False)
    OUT = nc.declare_dram_parameter("OUT", [P, T], BF16, isOutput=True)

    hoist_names = []

    with _SlimTileContext(nc) as tc, ExitStack() as ctx:
        singles = ctx.enter_context(tc.tile_pool(name="singles", bufs=1))
        ps_s = ctx.enter_context(tc.tile_pool(name="ps_s", bufs=1, space="PSUM"))
        ps_tp = ctx.enter_context(tc.tile_pool(name="ps_tp", bufs=1, space="PSUM"))
        ps_o = ctx.enter_context(tc.tile_pool(name="ps_o", bufs=1, space="PSUM"))

        # --- input tiles -----------------------------------------------
        q_sb = singles.tile([P, 2, 2, 8, HPC, HD], BF16)  # [r, x, c, h, kh]
        v_sb = singles.tile([P, NT, HPC, D], BF16)
        tb0_sb = singles.tile([P, 2 * 8 * HD + P], BF16)
        tb1_sb = singles.tile([P, 2 * 8 * HD], BF16)

        vv = v_sb  # [P, c, h, d]
        ii = [
            nc.scalar.dma_start(out=q_sb[:, 0], in_=QQ0[:]),
            nc.sync.dma_start(out=tb0_sb, in_=TB0[:]),
            nc.scalar.dma_start(out=q_sb[:, 1], in_=QQ1[:]),
            nc.sync.dma_start(out=tb1_sb, in_=TB1[:]),
            nc.sync.dma_start(out=vv[:, 0:8], in_=VV0[:]),
            nc.scalar.dma_start(out=vv[:, 8:16], in_=VV1[:]),
        ]
        hoist_names += [i.ins.name for i in ii]

        # --- early, dependency-free work -------------------------------
        s2d = singles.tile([P, P], BF16)
        nc.gpsimd.memset(s2d, 0.0)  # off-diagonal stays 0 for phase 3

        TW = 8 * HD  # 256: one table's cols
        q_r = [q_sb[:, 0], q_sb[:, 1]]
        idt = tb0_sb[:, 2 * TW :]
        sin_t = [
            tb0_sb[:, 0:TW].rearrange("p (c a k) -> p c a k", c=8, a=1),
            tb1_sb[:, 0:TW].rearrange("p (c a k) -> p c a k", c=8, a=1),
        ]
        cos_t = [
            tb0_sb[:, TW : 2 * TW].rearrange("p (c a k) -> p c a k", c=8, a=1),
            tb1_sb[:, TW : 2 * TW].rearrange("p (c a k) -> p c a k", c=8, a=1),
        ]

        # --- tiles -----------------------------------------------------
        cp = singles.tile([P, 2, 8, HPC, HD], BF16)  # [x, c, h, kh]
        sp = singles.tile([P, 2, 8, HPC, HD], BF16)
        # h-OUTER row space (h, x, kh): head h owns contiguous rows
        # h*64..h*64+64 of S2, so the s2d head-diagonal selection is two
        # rectangle copies instead of four strip copies.
        qr_r = singles.tile([P, NT, HPC, 2, HD], BF16)  # [c16, h, x, kh]
        # one qrt tile per evac unit: a single tile makes phase-3 block
        # g1 wait on the (unrelated) c8-15 evacs via tile-granular deps
        qrtA = singles.tile([P, 8 * P], BF16)
        qrtB1 = singles.tile([P, 4 * P], BF16)
        qrtB2 = singles.tile([P, 4 * P], BF16)
        outT_sb = singles.tile([P, T], BF16)

        s2_ps = ps_s.tile([P, P], F32)
        tpA = ps_tp.tile([P, 8 * P], BF16, tag="tpA", name="tpA")
        # two separate half-size tiles: a single tpB makes the scheduler
        # see a WAR between evac of c8-11 and the c12-15 transposes
        tpB1 = ps_tp.tile([P, 4 * P], BF16, tag="tpB1", name="tpB1")
        tpB2 = ps_tp.tile([P, 4 * P], BF16, tag="tpB2", name="tpB2")

        bshape = [P, 8, HPC, HD]

        def rope(r, subs):
            cosb = cos_t[r][:].to_broadcast(bshape)
            sinb = sin_t[r][:].to_broadcast(bshape)
            nc.vector.tensor_mul(sp[:, 0], q_r[r][:, 0], sinb)
            nc.vector.tensor_mul(cp[:, 0], q_r[r][:, 0], cosb)
            nc.vector.tensor_mul(sp[:, 1], q_r[r][:, 1], sinb)
            nc.vector.tensor_mul(cp[:, 1], q_r[r][:, 1], cosb)
            out = []
            for a, b in subs:
                cs = slice(r * 8 + a, r * 8 + b)
                ch = slice(a, b)
                out.append(
                    (
                        lambda cs=cs, ch=ch: (
                            nc.vector.tensor_sub(
                                qr_r[:, cs, :, 0], cp[:, 0, ch], sp[:, 1, ch]
                            ),
                            nc.vector.tensor_add(
                                qr_r[:, cs, :, 1], cp[:, 1, ch], sp[:, 0, ch]
                            ),
                        )
                    )
                )
            return out

        def qr2(c):
            # rows in (h, x, kh) order; the chunk slice is fully
            # contiguous, which the weights-AP verifier requires.
            return qr_r[:, c].rearrange("p h x k -> p (h x k)")

        def v2(c):
            return v_sb[:, c].rearrange("p h d -> p (h d)")

        # r0 RoPE, whole-range combine
        for fn in rope(0, [(0, 8)]):
            fn()
        # transposes c0-7 need only qr; V may still be streaming
        for c in range(8):
            nc.tensor.transpose(tpA[:, c * P : (c + 1) * P], qr2(c), idt)
        # ACT: one wide evac of chunks 0-7 (bf16 PSUM -> SBUF)
        nc.scalar.copy(out=qrtA[:], in_=tpA[:])
        # S-matmuls c0-7 (accumulating)
        for c in range(8):
            nc.tensor.matmul(
                s2_ps, lhsT=qr2(c), rhs=v2(c), start=(c == 0), stop=False
            )
        # r1 RoPE with split combines so S c8-11 can start early
        combines = rope(1, [(0, 4), (4, 8)])
        combines[0]()
        for c in range(8, 12):
            nc.tensor.matmul(s2_ps, lhsT=qr2(c), rhs=v2(c), start=False, stop=False)
        combines[1]()
        for c in range(12, 16):
            nc.tensor.matmul(
                s2_ps, lhsT=qr2(c), rhs=v2(c), start=False, stop=(c == 15)
            )

        # Head-diagonal blocks of S2 into the (pre-zeroed) phase-3
        # operand. With (h, x, kh) rows, head h owns the contiguous
        # rows AND cols h*64..h*64+64: two rectangle copies on DVE.
        # High priority pins them right after the stop S-matmul (the
        # list scheduler otherwise defers them behind the transposes'
        # evacs, delaying phase 3 by >1us).
        with tc.high_priority():
            nc.vector.tensor_copy(out=s2d[0:D, 0:D], in_=s2_ps[0:D, 0:D])
            nc.scalar.copy(out=s2d[D:, D:], in_=s2_ps[D:, D:])

        # transposes c8-15; evacs on DVE (bf16 copy runs 2x there vs
        # ACT, and ACT is the tail bottleneck with the phase-3 casts)
        for c in range(8, 12):
            nc.tensor.transpose(tpB1[:, (c - 8) * P : (c - 7) * P], qr2(c), idt)
        nc.vector.tensor_copy(out=qrtB1[:], in_=tpB1[:])
        for c in range(12, 16):
            nc.tensor.transpose(tpB2[:, (c - 12) * P : (c - 11) * P], qr2(c), idt)
        nc.vector.tensor_copy(out=qrtB2[:], in_=tpB2[:])

        # phase 3: blockdiag(S)^T @ QRT serves both heads at once.
        # Casts split DVE (low half) / ACT (high half); outputs leave as
        # two 1024-col DMAs so the packets are 2KB rows.
        qrt_blocks = [qrtA[:, 0:512], qrtA[:, 512:1024], qrtB1[:], qrtB2[:]]
        for g in range(4):
            o_ps = ps_o.tile([P, 512], F32, tag=f"o{g}", name=f"o{g}")
            blk = slice(g * 512, (g + 1) * 512)
            nc.tensor.matmul(
                o_ps, lhsT=s2d, rhs=qrt_blocks[g], start=True, stop=True
            )
            lo = slice(g * 512, g * 512 + 256)
            hi = slice(g * 512 + 256, (g + 1) * 512)
            nc.vector.tensor_copy(out=outT_sb[:, lo], in_=o_ps[:, 0:256])
            nc.scalar.copy(out=outT_sb[:, hi], in_=o_ps[:, 256:512])
            if g == 1:
                nc.sync.dma_start(out=OUT[:, 0:1024], in_=outT_sb[:, 0:1024])
            elif g == 3:
                nc.scalar.dma_start(out=OUT[:, 1024:2048], in_=outT_sb[:, 1024:2048])

    _split_multi_waits(nc)
    _hoist_input_dmas(nc, hoist_names)
    return nc


def _split_multi_waits(nc):
    """This compiler build rejects instructions carrying more than one
    sync-wait command; split extras into single-wait NoOps placed
    immediately before on the same engine."""
    n = 0
    for f in nc.m.functions:
        for blk in f.blocks:
            new_insts = []
            for inst in blk.instructions:
                si = inst.sync_info
                waits = list(si.on_wait) if si else []
                if len(waits) > 1:
                    for w in waits[:-1]:
                        nop = mybir.InstNoOp(name=f"W-split-{n}", ins=[], outs=[])
                        n += 1
                        nop.engine = inst.engine
                        nop.sync_info = mybir.SyncInfo(on_wait=[w], on_update=[])
                        new_insts.append(nop)
                    inst.sync_info = mybir.SyncInfo(
                        on_wait=[waits[-1]], on_update=list(si.on_update)
                    )
                new_insts.append(inst)
            blk.instructions = new_insts


def _hoist_input_dmas(nc, names):
    """Move the (dependency-free) input DMA issues from the kernel body
    to just before each engine's entry-barrier instruction in `main`, so
    the transfers are in flight during the tail of the fixed preamble.
    The DMA semaphores are runtime-zeroed before the NEFF starts and the
    consumers wait on absolute sem values, so only issue order matters;
    per-engine program order is preserved."""
    names = set(names)
    f = nc.m.functions[0]
    blocks = {b.name: b for b in f.blocks}
    main = blocks["main"]
    moved = []
    for b in f.blocks:
        if b.name == "main":
            continue
        keep = []
        for inst in b.instructions:
            if inst.name in names:
                si = inst.sync_info
                assert not (si and si.on_wait), f"hoisted DMA {inst.name} has waits"
                moved.append(inst)
            else:
                keep.append(inst)
        if len(keep) != len(b.instructions):
            b.instructions = keep
    assert len(moved) == len(names), (len(moved), names)
    # insert each engine's DMAs before its pre-barrier drain (the drain
    # + barrier pair costs ~1us on SP; the issue can precede it).
    # EXCEPT on Pool: the Pool drain waits for the SWDGE queue to go
    # EMPTY, so an issue placed before it stalls the drain (and the
    # all-engine barrier, and the whole body) for the entire transfer.
    # Pool issues go after the drain, right before the barrier inst.
    insert_at = {}
    pending_drain = {}
    for idx, inst in enumerate(main.instructions):
        if type(inst).__name__ == "InstDrain":
            pending_drain[inst.engine] = idx
        if inst.name.startswith("barrier_") and inst.engine not in insert_at:
            if inst.engine == mybir.EngineType.Pool:
                insert_at[inst.engine] = idx
            else:
                insert_at[inst.engine] = pending_drain.get(inst.engine, idx)
    new_main = []
    for idx, inst in enumerate(main.instructions):
        for eng, at in insert_at.items():
            if at == idx:
                new_main.extend(m for m in moved if m.engine == eng)
        new_main.append(inst)
    main.instructions = new_main


_NC_CACHE = None


def _get_nc():
    global _NC_CACHE
    if _NC_CACHE is None:
        _NC_CACHE = _build_nc()
    return _NC_CACHE


def _pack_inputs(Qs, Vs, cos32, sin32, idt):
    import ml_dtypes

    bf16 = ml_dtypes.bfloat16

    # [T, X] -> [P, NT, X] with t = p*NT + c
    def r(x):
        return x.reshape(P, NT, -1)

    ce = r(cos32).reshape(P, 2, 8, HD)  # [p, r, c, kh]
    se = r(sin32).reshape(P, 2, 8, HD)
    tb0 = np.ascontiguousarray(
        np.concatenate(
            [se[:, 0].reshape(P, -1), ce[:, 0].reshape(P, -1), idt], axis=1
        ).astype(bf16)
    )
    tb1 = np.ascontiguousarray(
        np.concatenate(
            [se[:, 1].reshape(P, -1), ce[:, 1].reshape(P, -1)], axis=1
        ).astype(bf16)
    )

    in_maps = []
    for core in range(N_CORES):
        h0 = core * HPC
        # q[p, r, x, c, h, kh], v[p, c, h, d]
        q = np.empty((P, 2, 2, 8, HPC, HD), np.float32)
        v = np.empty((P, NT, HPC, D), np.float32)
        for h in range(HPC):
            qh = r(Qs[h0 + h]).reshape(P, 2, 8, D)  # [p, r, c, d]
            q[:, :, 0, :, h] = qh[:, :, :, :HD]
            q[:, :, 1, :, h] = qh[:, :, :, HD:]
            v[:, :, h] = r(Vs[h0 + h])
        qb = q.astype(bf16)
        vb = v.astype(bf16)
        in_maps.append(
            {
                "TB0": tb0,
                "TB1": tb1,
                "QQ0": np.ascontiguousarray(qb[:, 0].reshape(P, -1)),
                "QQ1": np.ascontiguousarray(qb[:, 1].reshape(P, -1)),
                "VV0": np.ascontiguousarray(vb[:, 0:8].reshape(P, -1)),
                "VV1": np.ascontiguousarray(vb[:, 8:16].reshape(P, -1)),
            }
        )
    return in_maps


def _unpack_out(o):
    # o: [P, T] = outT; rows h*64+j, cols c-major: col = c*128 + f, t = f*16+c
    a = o.reshape(HPC, D, NT, P)  # [h, j, c, f]
    return a.transpose(0, 3, 2, 1).reshape(HPC, T, D)  # [h, t=f*16+c, j]


def run_inner(Q, K, V, trace=False):
    del K  # the module sets KR = QR; K is unused
    Qs = np.asarray(Q, dtype=np.float32)[0]  # [H, T, D]
    Vs = np.asarray(V, dtype=np.float32)[0]
    cos32, sin32 = _rope_tables()
    idt = np.eye(P, dtype=np.float32)
    nc = _get_nc()
    in_maps = _pack_inputs(Qs, Vs, cos32, sin32, idt)
    res = run_bass_kernel_spmd(nc, in_maps, list(range(N_CORES)), trace=trace)
    outs = [_unpack_out(np.asarray(res.results[i]["OUT"])) for i in range(N_CORES)]
    out = np.concatenate(outs, axis=0)[None]  # [1, H, T, D]
    return out.astype(np.float32), res


def kernel(Q, K, V):
    out, _ = run_inner(Q, K, V, trace=False)
    return out
